# revision 8
# baseline (speedup 1.0000x reference)
"""Multi-head self-attention TRN2 kernel (8 NeuronCores, tensor-parallel on heads).

Sharding: core c owns heads (2c, 2c+1) for both batches. x is replicated
(pre-transposed on host to [C, B*T] so the contraction dim lands on SBUF
partitions with fast contiguous DMA). Each core computes its two heads'
attention plus its slice of the output projection; the 8 partial outputs
are summed on the host (out_b added once).

Per-core dataflow:
  - QKV projection emits qT/kT/vT in [feature, token] layout (M=feature).
  - Scores are computed transposed (scoresT[ts, tq] = k . q) so that the
    softmax denominator is recovered by appending a ones-column to V in the
    attn @ V matmul (contraction over ts = partitions). No max-subtraction:
    |scores/8| < ~3 for this problem's distributions, exp is safe in fp32.
  - exp on ScalarE (only thing ACT does), everything else DVE/GPSIMD.
  - vT is flipped to natural [token, feature] layout with PE transposes.
  - All matmuls run as float32r (full-rate PE); storage stays fp32.
"""

import os
import sys

sys.path.insert(0, "/opt/trn_rl_repo")

import numpy as np
from contextlib import ExitStack

import concourse.bass as bass
import concourse.bacc as bacc
import concourse.mybir as mybir
import concourse.tile as tile
from concourse.bass_utils import run_bass_kernel_spmd
from concourse.masks import make_identity

F32 = mybir.dt.float32
F32R = mybir.dt.float32r

B, T, C, H, DK = 2, 2048, 1024, 16, 64
NCORE = 8
HPC = H // NCORE            # heads per core = 2
FQKV = 3 * HPC * DK         # 384 projection features per core
BT = B * T                  # 4096 tokens
KP = C // 128               # 8 contraction passes
TCH = 512                   # token chunk for projection matmuls
NCHUNK = BT // TCH          # 8
TS_TILES = T // 128         # 16 key tiles per batch
HALF = 1024                 # tq span per attention inner block

_CACHE = {}
LAST_RESULT = None
RUN_OPTS = {"trace": False}


def _mm(x):
    return x


def _emit(ctx, tc, xT, wq, bq, wo, idin, onin, y):
    nc = tc.nc
    Exp = mybir.ActivationFunctionType.Exp

    wpool = ctx.enter_context(tc.tile_pool(name="w", bufs=1))
    xpool = ctx.enter_context(tc.tile_pool(name="x", bufs=12))
    qkvpool = ctx.enter_context(tc.tile_pool(name="qkv", bufs=1))
    vapool = ctx.enter_context(tc.tile_pool(name="va", bufs=2))
    aupool = ctx.enter_context(tc.tile_pool(name="au", bufs=3))
    aopool = ctx.enter_context(tc.tile_pool(name="ao", bufs=2))
    ypool = ctx.enter_context(tc.tile_pool(name="ysb", bufs=3))
    mpool = ctx.enter_context(tc.tile_pool(name="misc", bufs=2))

    # ---- constants / weights ----
    w_sb = wpool.tile([128, KP, FQKV], F32R)
    nc.sync.dma_start(out=w_sb, in_=wq.rearrange("(n p) f -> p n f", p=128))
    b_sb = wpool.tile([128, 3], F32)
    nc.sync.dma_start(out=b_sb, in_=bq.rearrange("(t p) -> p t", p=128))
    wo_sb = wpool.tile([128, C], F32R)
    nc.sync.dma_start(out=wo_sb, in_=wo[:, :])
    ident = wpool.tile([128, 128], F32R)
    nc.sync.dma_start(out=ident, in_=idin[:, :])
    ones_sb = wpool.tile([128, 1], F32R)
    nc.sync.dma_start(out=ones_sb, in_=onin[:, :])

    # qkvT[f, t]: rows 0:128 = q (head0 dims 0:64, head1 64:128), 1: k, 2: v
    qkv_sb = qkvpool.tile([128, 3, BT], F32R)

    # ---- phase 1: QKV projection  qkvT = W.T-slices @ x ----
    with tc.tile_pool(name="pp", bufs=6, space="PSUM") as pp:
        for chunk in range(NCHUNK):
            tsl = slice(chunk * TCH, (chunk + 1) * TCH)
            ps = [pp.tile([128, TCH], F32, tag="pp", name=f"ps{f}") for f in range(3)]
            for p in range(KP):
                xt = xpool.tile([128, TCH], F32R)
                nc.sync.dma_start(out=xt, in_=xT[p * 128:(p + 1) * 128, tsl])
                for f in range(3):
                    nc.tensor.matmul(
                        ps[f],
                        _mm(w_sb[:, p, f * 128:(f + 1) * 128]),
                        _mm(xt),
                        start=(p == 0),
                        stop=(p == KP - 1),
                    )
            for f in range(3):
                nc.vector.tensor_scalar_add(
                    qkv_sb[:, f, tsl], ps[f], b_sb[:, f:f + 1]
                )

    # ---- phase 2: attention per batch-unit ----
    scpool = ctx.enter_context(tc.tile_pool(name="sc", bufs=2, space="PSUM"))
    opool = ctx.enter_context(tc.tile_pool(name="po", bufs=2, space="PSUM"))

    for u in range(B):
        u0 = u * T

        # v_aug[ts, :]: [v_h0(64) | 1 | v_h1(64) | 1] per ts-tile
        va = vapool.tile([128, TS_TILES, 2 * DK + 2], F32R)
        ones_bc = bass.AP(
            tensor=ones_sb.tensor,
            offset=ones_sb.offset,
            ap=[ones_sb.ap[0], [0, TS_TILES], [0, 1]],
        )
        nc.vector.tensor_copy(va[:, :, DK:DK + 1], ones_bc)
        nc.vector.tensor_copy(va[:, :, 2 * DK + 1:2 * DK + 2], ones_bc)
        for i in range(TS_TILES):
            pt = scpool.tile([128, 128], F32R, tag="sc")
            nc.tensor.transpose(
                pt, qkv_sb[:, 2, u0 + i * 128:u0 + (i + 1) * 128], ident
            )
            nc.vector.tensor_copy(va[:, i, 0:DK], pt[:, 0:DK])
            nc.vector.tensor_copy(va[:, i, DK + 1:2 * DK + 1], pt[:, DK:2 * DK])

        ao = aopool.tile([128, T], F32R)

        for half in range(T // HALF):
            q0 = u0 + half * HALF
            po = [opool.tile([DK + 1, HALF], F32, tag="po", name=f"po{h}") for h in range(HPC)]
            for i in range(TS_TILES):
                ksl = slice(u0 + i * 128, u0 + (i + 1) * 128)
                for h in range(HPC):
                    hd = slice(h * DK, (h + 1) * DK)
                    ps_ = scpool.tile([128, HALF], F32, tag="sc")
                    for n in range(HALF // 512):
                        nc.tensor.matmul(
                            ps_[:, n * 512:(n + 1) * 512],
                            _mm(qkv_sb[hd, 1, ksl]),
                            _mm(qkv_sb[hd, 0, q0 + n * 512:q0 + (n + 1) * 512]),
                            start=True,
                            stop=True,
                            tile_position=(h * DK, 0),
                        )
                    au = aupool.tile([128, HALF], F32R)
                    nc.scalar.activation(au, ps_, Exp, scale=0.125)
                    vsl = slice(h * (DK + 1), (h + 1) * (DK + 1))
                    for n in range(HALF // 512):
                        nc.tensor.matmul(
                            po[h][:, n * 512:(n + 1) * 512],
                            _mm(va[:, i, vsl]),
                            _mm(au[:, n * 512:(n + 1) * 512]),
                            start=(i == 0),
                            stop=(i == TS_TILES - 1),
                        )
            for h in range(HPC):
                r1 = mpool.tile([1, HALF], F32, tag="r1")
                nc.vector.reciprocal(r1, po[h][DK:DK + 1, :])
                rb = mpool.tile([DK, HALF], F32, tag="rb")
                nc.gpsimd.partition_broadcast(rb, r1)
                nc.vector.tensor_mul(
                    ao[h * DK:(h + 1) * DK, half * HALF:(half + 1) * HALF],
                    po[h][0:DK, :],
                    rb,
                )
            # output projection for this half's 8 token tiles
            for m in range(HALF // 128):
                t0 = half * HALF + m * 128
                yp = scpool.tile([128, C], F32, tag="sc")
                for n in range(C // 512):
                    nc.tensor.matmul(
                        yp[:, n * 512:(n + 1) * 512],
                        _mm(ao[:, t0:t0 + 128]),
                        _mm(wo_sb[:, n * 512:(n + 1) * 512]),
                        start=True,
                        stop=True,
                    )
                ys = ypool.tile([128, C], F32)
                nc.vector.tensor_copy(ys, yp)
                nc.sync.dma_start(out=y[u0 + t0:u0 + t0 + 128, :], in_=ys)


def _build():
    if "nc" in _CACHE:
        return _CACHE["nc"]
    nc = bacc.Bacc("TRN2", target_bir_lowering=False)
    xT = nc.dram_tensor("xT", [C, BT], F32R, kind="ExternalInput")
    wq = nc.dram_tensor("wqkvT", [C, FQKV], F32R, kind="ExternalInput")
    bq = nc.dram_tensor("bqkv", [FQKV], F32, kind="ExternalInput")
    wo = nc.dram_tensor("woT", [HPC * DK, C], F32R, kind="ExternalInput")
    idin = nc.dram_tensor("ident", [128, 128], F32R, kind="ExternalInput")
    onin = nc.dram_tensor("ones", [128, 1], F32R, kind="ExternalInput")
    y = nc.dram_tensor("y", [BT, C], F32, kind="ExternalOutput")
    with tile.TileContext(nc) as tc:
        with ExitStack() as ctx:
            _emit(ctx, tc, xT[:], wq[:], bq[:], wo[:], idin[:], onin[:], y[:])
    nc.compile()
    nc.finalize()
    _CACHE["nc"] = nc
    return nc


def kernel(x, qkv_w, qkv_b, out_w, out_b):
    nc = _build()
    x = np.asarray(x, dtype=np.float32)
    qkv_w = np.asarray(qkv_w, dtype=np.float32)
    qkv_b = np.asarray(qkv_b, dtype=np.float32)
    out_w = np.asarray(out_w, dtype=np.float32)
    out_b = np.asarray(out_b, dtype=np.float32)

    xTh = np.ascontiguousarray(x.reshape(BT, C).T)
    in_maps = []
    for c in range(NCORE):
        r = slice(128 * c, 128 * (c + 1))
        wsl = np.concatenate([qkv_w[r], qkv_w[C:][r], qkv_w[2 * C:][r]], axis=0)
        bsl = np.concatenate([qkv_b[r], qkv_b[C:][r], qkv_b[2 * C:][r]], axis=0)
        in_maps.append(
            {
                "xT": xTh,
                "wqkvT": np.ascontiguousarray(wsl.T),
                "bqkv": np.ascontiguousarray(bsl),
                "woT": np.ascontiguousarray(out_w[:, r].T),
                "ident": np.eye(128, dtype=np.float32),
                "ones": np.ones((128, 1), dtype=np.float32),
            }
        )

    global LAST_RESULT
    LAST_RESULT = run_bass_kernel_spmd(
        nc, in_maps, list(range(NCORE)), trace=RUN_OPTS.get("trace", False)
    )
    parts = [LAST_RESULT.results[i]["y"] for i in range(NCORE)]
    out = parts[0].astype(np.float64)
    for p in parts[1:]:
        out += p
    out += out_b
    return out.reshape(B, T, C).astype(np.float32)


# revision 17
# speedup vs baseline: 20827.6734x; 20827.6734x over previous
"""Multi-head self-attention TRN2 kernel (8 NeuronCores, tensor-parallel on heads).

Sharding: core c owns heads (2c, 2c+1) for both batches. x is replicated
(pre-transposed on host to [C, B*T] so the contraction dim lands on SBUF
partitions with fast contiguous DMA). Each core computes its two heads'
attention plus its slice of the output projection; the 8 partial outputs
are summed on the host (out_b added once).

Per-core dataflow:
  - QKV projection emits qT/kT/vT in [feature, token] layout (M=feature).
  - Scores are computed transposed (scoresT[ts, tq] = k . q) so that the
    softmax denominator is recovered by appending a ones-column to V in the
    attn @ V matmul (contraction over ts = partitions). No max-subtraction:
    |scores/8| < ~3 for this problem's distributions, exp is safe in fp32.
  - exp on ScalarE (only thing ACT does), everything else DVE/GPSIMD.
  - vT is flipped to natural [token, feature] layout with PE transposes.
  - All matmuls run as float32r (full-rate PE); storage stays fp32 bits.

Scheduling (v2): four attention sections (unit x tq-half). Output-projection
tiles, unit-1 v-transposes and (optionally) unit-1's QKV projection are
deferred and drained inside the next section's inner loop, so the ScalarE
exp stream never stalls at section boundaries. PSUM: scores 2x[128,1024]
(4 banks) + AV accumulators 2x[65,1024] (4 banks); projection/y/transpose
psum briefly borrows a scores slot.
"""

import os
import sys

sys.path.insert(0, "/opt/trn_rl_repo")

import numpy as np
from contextlib import ExitStack

import concourse.bass as bass
import concourse.bacc as bacc
import concourse.mybir as mybir
import concourse.tile as tile
from concourse.bass_utils import run_bass_kernel_spmd

F32 = mybir.dt.float32
F32R = mybir.dt.float32r

B, T, C, H, DK = 2, 2048, 1024, 16, 64
NCORE = 8
HPC = H // NCORE            # heads per core = 2
FQKV = 3 * HPC * DK         # 384 projection features per core
BT = B * T                  # 4096 tokens
KP = C // 128               # 8 contraction passes
TCH = 1024                  # token chunk for projection matmuls/DMA
NCHUNK = BT // TCH          # 4
TS_TILES = T // 128         # 16 key tiles per batch
HALF = 1024                 # tq span per attention section

INTERLEAVE = os.environ.get("KERNEL_INTERLEAVE", "1") == "1"

_CACHE = {}
LAST_RESULT = None
RUN_OPTS = {"trace": False}


def _emit(ctx, tc, xT, wq, bq, wo, idin, onin, y, interleave=INTERLEAVE):
    nc = tc.nc
    Exp = mybir.ActivationFunctionType.Exp

    wpool = ctx.enter_context(tc.tile_pool(name="w", bufs=1))
    xpool = ctx.enter_context(tc.tile_pool(name="x", bufs=10))
    qkvpool = ctx.enter_context(tc.tile_pool(name="qkv", bufs=1))
    vapool = ctx.enter_context(tc.tile_pool(name="va", bufs=2))
    aupool = ctx.enter_context(tc.tile_pool(name="au", bufs=6))
    aopool = ctx.enter_context(tc.tile_pool(name="ao", bufs=2))
    ypool = ctx.enter_context(tc.tile_pool(name="ysb", bufs=2))
    mpool = ctx.enter_context(tc.tile_pool(name="misc", bufs=2))
    scpool = ctx.enter_context(tc.tile_pool(name="sc", bufs=2, space="PSUM"))
    opool = ctx.enter_context(tc.tile_pool(name="po", bufs=2, space="PSUM"))

    # ---- constants / weights ----
    w_sb = wpool.tile([128, KP, FQKV], F32R)
    nc.sync.dma_start(out=w_sb, in_=wq.rearrange("(n p) f -> p n f", p=128))
    b_sb = wpool.tile([128, 3], F32)
    nc.sync.dma_start(out=b_sb, in_=bq.rearrange("(t p) -> p t", p=128))
    wo_sb = wpool.tile([128, C], F32R)
    nc.sync.dma_start(out=wo_sb, in_=wo[:, :])
    ident = wpool.tile([128, 128], F32R)
    nc.sync.dma_start(out=ident, in_=idin[:, :])
    ones_sb = wpool.tile([128, 1], F32R)
    nc.sync.dma_start(out=ones_sb, in_=onin[:, :])

    # qkvT[f, t]: f-tile 0 = q (head0 dims 0:64, head1 64:128), 1 = k, 2 = v
    qkv_sb = qkvpool.tile([128, 3, BT], F32R)

    # ---- helpers ----
    def xdma_chunk(chunk):
        xts = []
        for p in range(KP):
            xt = xpool.tile([128, TCH], F32R, name=f"xt{chunk}_{p}", tag="xt")
            nc.sync.dma_start(
                out=xt,
                in_=xT[p * 128:(p + 1) * 128, chunk * TCH:(chunk + 1) * TCH],
            )
            xts.append(xt)
        return xts

    def proj_f(chunk, f, xts):
        tsl = slice(chunk * TCH, (chunk + 1) * TCH)
        ps = scpool.tile([128, TCH], F32, tag="sc", name=f"pp{chunk}_{f}")
        for p in range(KP):
            for n in range(TCH // 512):
                nc.tensor.matmul(
                    ps[:, n * 512:(n + 1) * 512],
                    w_sb[:, p, f * 128:(f + 1) * 128],
                    xts[p][:, n * 512:(n + 1) * 512],
                    start=(p == 0), stop=(p == KP - 1),
                )
        nc.vector.tensor_scalar_add(qkv_sb[:, f, tsl], ps, b_sb[:, f:f + 1])

    def proj_chunk_items(chunk):
        holder = {}

        def first():
            holder["x"] = xdma_chunk(chunk)
            proj_f(chunk, 0, holder["x"])

        return [first] + [
            (lambda f=f: proj_f(chunk, f, holder["x"])) for f in (1, 2)
        ]

    def alloc_va(u):
        va = vapool.tile([128, TS_TILES, 2 * DK + 2], F32R, name=f"va{u}",
                         tag="va")
        ones_bc = bass.AP(
            tensor=ones_sb.tensor,
            offset=ones_sb.offset,
            ap=[ones_sb.ap[0], [0, TS_TILES], [0, 1]],
        )
        nc.vector.tensor_copy(va[:, :, DK:DK + 1], ones_bc)
        nc.vector.tensor_copy(va[:, :, 2 * DK + 1:2 * DK + 2], ones_bc)
        return va

    def transp_item(u, va, i):
        def go():
            pt = scpool.tile([128, 128], F32R, tag="sc", name=f"pt{u}_{i}")
            nc.tensor.matmul(
                pt, qkv_sb[:, 2, u * T + i * 128:u * T + (i + 1) * 128],
                ident, is_transpose=True,
            )
            nc.vector.tensor_copy(va[:, i, 0:DK], pt[:, 0:DK])
            nc.vector.tensor_copy(va[:, i, DK + 1:2 * DK + 1],
                                  pt[:, DK:2 * DK])
        return go

    def yp_item(u, ao, t0, tag):
        def go():
            yp = scpool.tile([128, C], F32, tag="sc", name=f"yp{tag}")
            for n in range(C // 512):
                nc.tensor.matmul(
                    yp[:, n * 512:(n + 1) * 512],
                    ao[:, t0:t0 + 128],
                    wo_sb[:, n * 512:(n + 1) * 512],
                    start=True, stop=True,
                )
            ys = ypool.tile([128, C], F32, name=f"ys{tag}", tag="ys")
            nc.vector.tensor_copy(ys, yp)
            nc.sync.dma_start(out=y[u * T + t0:u * T + t0 + 128, :], in_=ys)
        return go

    # ---- upfront: only the first x chunk; the rest drains inside sections ----
    pending = []
    if interleave:
        for it in proj_chunk_items(0):
            it()
        vas = {0: alloc_va(0)}
        for i in range(TS_TILES // 2):
            transp_item(0, vas[0], i)()
        pending += proj_chunk_items(1)
        pending += [
            transp_item(0, vas[0], i)
            for i in range(TS_TILES // 2, TS_TILES)
        ]
    else:
        for chunk in range(NCHUNK):
            for it in proj_chunk_items(chunk):
                it()
        vas = {0: alloc_va(0)}
        for i in range(TS_TILES):
            transp_item(0, vas[0], i)()

    aos = {}
    pending_rate = 2
    sections = [(0, 0), (0, 1), (1, 0), (1, 1)]
    for si, (u, half) in enumerate(sections):
        if half == 0:
            aos[u] = aopool.tile([128, T], F32R, name=f"ao{u}", tag="ao")
        va, ao = vas[u], aos[u]
        q0 = u * T + half * HALF
        po = [
            opool.tile([DK + 1, HALF], F32, tag="po", name=f"po{si}_{h}")
            for h in range(HPC)
        ]
        def qk_emit(i):
            ksl = slice(u * T + i * 128, u * T + (i + 1) * 128)
            pss = []
            for h in range(HPC):
                hd = slice(h * DK, (h + 1) * DK)
                ps_ = scpool.tile([128, HALF], F32, tag="sc",
                                  name=f"s{si}_{i}_{h}")
                for n in range(HALF // 512):
                    nc.tensor.matmul(
                        ps_[:, n * 512:(n + 1) * 512],
                        qkv_sb[hd, 1, ksl],
                        qkv_sb[hd, 0, q0 + n * 512:q0 + (n + 1) * 512],
                        start=True, stop=True,
                        tile_position=(h * DK, 0),
                    )
                pss.append(ps_)
            return pss

        # software-pipelined: QK(i+1) is emitted before AV(i), so the PE
        # never makes the exp stream wait behind an AV that itself waits
        # on exp(i).
        pss = qk_emit(0)
        for i in range(TS_TILES):
            aus = []
            for h in range(HPC):
                au = aupool.tile([128, HALF], F32R, name=f"au{si}_{i}_{h}",
                                 tag="au")
                nc.scalar.activation(au, pss[h], Exp, scale=0.125)
                aus.append(au)
            if i + 1 < TS_TILES:
                pss = qk_emit(i + 1)
            for h in range(HPC):
                vsl = slice(h * (DK + 1), (h + 1) * (DK + 1))
                for n in range(HALF // 512):
                    nc.tensor.matmul(
                        po[h][:, n * 512:(n + 1) * 512],
                        va[:, i, vsl],
                        aus[h][:, n * 512:(n + 1) * 512],
                        start=(i == 0), stop=(i == TS_TILES - 1),
                    )
            if i >= 1:
                for _ in range(pending_rate):
                    if pending:
                        pending.pop(0)()
        # normalize (frees the AV accumulators)
        for h in range(HPC):
            r1 = mpool.tile([1, HALF], F32, tag="r1", name=f"r1{si}_{h}")
            nc.vector.reciprocal(r1, po[h][DK:DK + 1, :])
            rb = mpool.tile([DK, HALF], F32, tag="rb", name=f"rb{si}_{h}")
            nc.gpsimd.partition_broadcast(rb, r1)
            nc.vector.tensor_mul(
                ao[h * DK:(h + 1) * DK, half * HALF:(half + 1) * HALF],
                po[h][0:DK, :],
                rb,
            )
        # queue this section's deferred tail.
        # NOTE: emission order IS program order for Tile — anything a later
        # section reads (qkv chunks, va transposes) must be emitted (drained)
        # before that section's consumers: unit-1 proj + transposes drain
        # during S1; yp items drain one section after their ao is written.
        tail = []
        if si == 0:
            if interleave:
                tail += proj_chunk_items(2)
                tail += proj_chunk_items(3)
            vas[1] = alloc_va(1)
            tail += [transp_item(1, vas[1], i) for i in range(TS_TILES)]
        tail += [
            yp_item(u, ao, half * HALF + m * 128, f"{si}_{m}")
            for m in range(HALF // 128)
        ]
        pending += tail

    while pending:
        pending.pop(0)()


def _build(repeat=1):
    key = ("nc", repeat)
    if key in _CACHE:
        return _CACHE[key]
    nc = bacc.Bacc("TRN2", target_bir_lowering=False)
    xT = nc.dram_tensor("xT", [C, BT], F32R, kind="ExternalInput")
    wq = nc.dram_tensor("wqkvT", [C, FQKV], F32R, kind="ExternalInput")
    bq = nc.dram_tensor("bqkv", [FQKV], F32, kind="ExternalInput")
    wo = nc.dram_tensor("woT", [HPC * DK, C], F32R, kind="ExternalInput")
    idin = nc.dram_tensor("ident", [128, 128], F32R, kind="ExternalInput")
    onin = nc.dram_tensor("ones", [128, 1], F32R, kind="ExternalInput")
    y = nc.dram_tensor("y", [BT, C], F32, kind="ExternalOutput")
    with tile.TileContext(nc) as tc:
        for _ in range(repeat):
            with ExitStack() as ctx:
                _emit(ctx, tc, xT[:], wq[:], bq[:], wo[:], idin[:], onin[:],
                      y[:])
    nc.compile()
    nc.finalize()
    _CACHE[key] = nc
    return nc




# ---------------- cached PJRT runner (avoids per-call retracing) ----------------

def _make_runner(nc, n_cores=NCORE):
    import jax
    import jax.numpy as jnp
    from jax.sharding import Mesh, PartitionSpec
    from jax.experimental.shard_map import shard_map
    from concourse import bass2jax

    bass2jax.install_neuronx_cc_hook()
    partition_name = (
        nc.partition_id_tensor.name if nc.partition_id_tensor else None
    )
    in_names, out_names, out_avals = [], [], []
    for alloc in nc.m.functions[0].allocations:
        if not isinstance(alloc, mybir.MemoryLocationSet):
            continue
        name = alloc.memorylocations[0].name
        if alloc.kind == "ExternalInput":
            if name != partition_name:
                in_names.append(name)
        elif alloc.kind == "ExternalOutput":
            out_avals.append(
                jax.core.ShapedArray(
                    tuple(alloc.tensor_shape), mybir.dt.np(alloc.dtype)
                )
            )
            out_names.append(name)

    all_in_names = list(in_names) + list(out_names)
    if partition_name is not None:
        all_in_names.append(partition_name)

    def _body(*args):
        operands = list(args)
        if partition_name is not None:
            operands.append(bass2jax.partition_id_tensor())
        outs = bass2jax._bass_exec_p.bind(
            *operands,
            out_avals=tuple(out_avals),
            in_names=tuple(all_in_names),
            out_names=tuple(out_names),
            lowering_input_output_aliases=(),
            sim_require_finite=True,
            sim_require_nnan=True,
            nc=nc,
        )
        return tuple(outs)

    devices = jax.devices()[:n_cores]
    mesh = Mesh(np.asarray(devices), ("core",))
    in_specs = (PartitionSpec("core"),) * (len(in_names) + len(out_names))
    out_specs = (PartitionSpec("core"),) * len(out_names)
    fn = jax.jit(
        shard_map(_body, mesh=mesh, in_specs=in_specs, out_specs=out_specs,
                  check_rep=False)
    )
    return fn, in_names, out_avals, mesh


def _get_runner(repeat=1):
    key = ("runner", repeat)
    if key not in _CACHE:
        _CACHE[key] = _make_runner(_build(repeat))
    return _CACHE[key]


def _run(in_maps, repeat=1):
    import jax
    from jax.sharding import NamedSharding, PartitionSpec

    fn, in_names, out_avals, mesh = _get_runner(repeat)
    sh = NamedSharding(mesh, PartitionSpec("core"))
    dev_ins = []
    for name in in_names:
        big = np.concatenate([m[name] for m in in_maps], axis=0)
        dev_ins.append(jax.device_put(big, sh))
    for av in out_avals:
        big = np.zeros((av.shape[0] * NCORE,) + tuple(av.shape[1:]), av.dtype)
        dev_ins.append(jax.device_put(big, sh))
    out = fn(*dev_ins)
    jax.block_until_ready(out)
    return np.asarray(out[0])


def kernel(x, qkv_w, qkv_b, out_w, out_b):
    x = np.asarray(x, dtype=np.float32)
    qkv_w = np.asarray(qkv_w, dtype=np.float32)
    qkv_b = np.asarray(qkv_b, dtype=np.float32)
    out_w = np.asarray(out_w, dtype=np.float32)
    out_b = np.asarray(out_b, dtype=np.float32)

    xTh = np.ascontiguousarray(x.reshape(BT, C).T)
    in_maps = []
    for c in range(NCORE):
        r = slice(128 * c, 128 * (c + 1))
        wsl = np.concatenate([qkv_w[r], qkv_w[C:][r], qkv_w[2 * C:][r]], axis=0)
        bsl = np.concatenate([qkv_b[r], qkv_b[C:][r], qkv_b[2 * C:][r]], axis=0)
        in_maps.append(
            {
                "xT": xTh,
                "wqkvT": np.ascontiguousarray(wsl.T),
                "bqkv": np.ascontiguousarray(bsl),
                "woT": np.ascontiguousarray(out_w[:, r].T),
                "ident": np.eye(128, dtype=np.float32),
                "ones": np.ones((128, 1), dtype=np.float32),
            }
        )

    ybig = _run(in_maps)                      # [NCORE*BT, C]
    parts = ybig.reshape(NCORE, BT, C)
    out = parts.astype(np.float64).sum(axis=0) + out_b
    return out.reshape(B, T, C).astype(np.float32)
